# revision 26
# baseline (speedup 1.0000x reference)
"""MinHash (segment-min of (a*x+b) mod P over uniform CSR segments) on 8 trn2 cores.

Strategy
--------
P = 2^31-1, K = 128 hashes, ids < 2^20, segments are uniform 64-wide (validated;
general inputs fall back to an exact host path).

Per (element i, hash j) we need h = (a_j*x_i + b_j) mod P, then min over each
64-element segment. Products reach 2^51, beyond fp32/int32, so x is split into
five 4-bit digits d_l and the per-hash coefficients c_lj = centered(a_j*16^l mod P)
are precomputed on host.  A single PE matmul per tile computes
    T~_ij = sum_l d_l(i)*c_lj + b_j   (|T| < 2^36, fp32 MAC error ~2^15)
which satisfies T == a*x+b (mod P).  The DVE then computes the *centered*
remainder r = T~ - 2^31*rne(T~/2^31) exactly via the round-to-ulp trick
(add/sub 1.5*2^54), so the only quotient-boundary ambiguity sits at h ~ 2^30 —
values that can never win a min.  Mapping r -> segment-min key is free:
reinterpret the fp32 bits as uint32 (negative r == huge h sorts above all
positive r) and take a grouped reduce_min.

True h = r + q with |q| <= 25, so device results are within ~2^16 absolute
(1e-5 relative on the 2^31 scale) everywhere; entries whose winning value is
tiny (< MARGIN, where exactness matters most and fp32 ordering could slip) are
recomputed exactly on host from the raw inputs (~1% of entries).
"""

import numpy as np

P = 2147483647
K = 128
SEG = 64
NCORES = 8
S_TOTAL = 100000
NNZ = 6400000

S_CORE = S_TOTAL // NCORES          # 12500 sets per core
E_CORE = NNZ // NCORES              # 800000 elements per core
SUB = 512                           # elements per matmul (one PSUM bank)
CHUNK = 4                           # matmuls per DVE chunk
CW = CHUNK * SUB                    # 2048 elements per chunk
NCHUNK = (E_CORE + CW - 1) // CW    # 391 chunks
CAP = NCHUNK * CW                   # 800768 padded elements
OUTC = CAP // SEG                   # 12512 output columns per core
OGRP = 16                           # chunks per output DMA (16*32 = 512 cols)
C_RNE = float(3 << 53)              # 1.5*2^54: fp32 add/sub rounds to mult of 2^31
MARGIN = 1 << 18                    # host-rescue threshold on winning value

_prog_cache = {}


def _build_program():
    """Raw-Bass program: matmul digits->T, centered remainder, grouped min.

    Hand-scheduled pipeline over four engine programs (gpsimd: input DMA,
    PE: matmuls, DVE: remainder+reduce, SP: output DMA) with explicit
    semaphores.  This walrus build allows at most one attached sync-wait per
    Matmult/DMACopy, so all waits are standalone sequencer instructions.
    """
    import concourse.bass as bass
    from concourse import mybir

    f32 = mybir.dt.float32
    nc = bass.Bass()
    xl = nc.dram_tensor("xl", [NCHUNK, 6, CW], f32, kind="ExternalInput")
    cw = nc.dram_tensor("cw", [6, K], f32, kind="ExternalInput")
    omin = nc.dram_tensor("omin", [K, OUTC], f32, kind="ExternalOutput")

    NGRP = (NCHUNK + OGRP - 1) // OGRP          # 25 output groups
    GC = CW // SEG                              # 32 reduce groups per chunk

    with (
        nc.sbuf_tensor("cw_s", [6, K], f32) as cw_s,
        nc.sbuf_tensor("x_sb", [6, 3, CW], f32) as x_sb,
        nc.psum_tensor("ps", [128, 8, SUB], f32) as ps,
        nc.sbuf_tensor("t_sb", [128, 2, CW], f32) as t_sb,
        nc.sbuf_tensor("u_sb", [128, 2, CW], f32) as u_sb,
        nc.sbuf_tensor("r_sb", [128, 2, CW], f32) as r_sb,
        nc.sbuf_tensor("acc", [128, 2, OGRP * GC], f32) as acc,
        nc.semaphore("s_in") as s_in,    # input DMAs done        (+16 each)
        nc.semaphore("s_pe") as s_pe,    # chunk matmuls done     (+1 per chunk)
        nc.semaphore("s_act") as s_act,  # ACT round pass done    (+1)
        nc.semaphore("s_gps") as s_gps,  # GPS subtract done      (+1)
        nc.semaphore("s_dvr") as s_dvr,  # DVE remainder done     (+1)
        nc.semaphore("s_red") as s_red,  # DVE reduce done        (+1)
        nc.semaphore("s_out") as s_out,  # output DMAs done       (+16 each)
        nc.Block() as block,
    ):
        @block.sync
        def _(s):
            # interleave input and output DMA issue on the SP sequencer; an
            # output for group g is issued once inputs are ~2 groups ahead so
            # its s_red wait never stalls input issue.
            s.dma_start(cw_s[:, :], cw[:, :]).then_inc(s_in, 16)
            gi = 0

            def emit_out(g):
                done = min((g + 1) * OGRP, NCHUNK)
                ncols = (done - g * OGRP) * GC
                if g >= 1:
                    s.wait_ge(s_out, 16 * g)
                s.wait_ge(s_red, done)
                s.dma_start(
                    omin[:, g * OGRP * GC: g * OGRP * GC + ncols],
                    acc[:, g % 2, 0:ncols],
                ).then_inc(s_out, 16)

            for ch in range(NCHUNK):
                if ch >= 1:  # sim race rule: prior issued DMAs must be done
                    s.wait_ge(s_in, 16 * (ch + 1))
                if ch >= 3:  # x slot reuse: PE done with ch-3
                    s.wait_ge(s_pe, ch - 2)
                s.dma_start(
                    x_sb[:, ch % 3, :], xl[ch, :, :]).then_inc(s_in, 16)
                if gi < NGRP and ch >= (gi + 2) * OGRP + 4:
                    emit_out(gi)
                    gi += 1
            while gi < NGRP:
                emit_out(gi)
                gi += 1

        @block.tensor
        def _(t):
            for ch in range(NCHUNK):
                t.wait_ge(s_in, 16 * (ch + 2))
                if ch >= 2:  # psum parity released by ACT & DVE readers
                    t.wait_ge(s_act, ch - 1)
                    t.wait_ge(s_dvr, ch - 1)
                p = 4 * (ch % 2)
                for k2 in range(CHUNK):
                    mm = nc.tensor.matmul(
                        ps[:, p + k2, :], lhsT=cw_s[:, :],
                        rhs=x_sb[:, ch % 3, k2 * SUB:(k2 + 1) * SUB],
                        start=True, stop=True,
                    )
                    if k2 == CHUNK - 1:
                        mm.then_inc(s_pe, 1)

        @block.scalar
        def _(a):
            # t1 = T~ + C  (fp32 RNE rounds to a multiple of 2^31 + C)
            for ch in range(NCHUNK):
                a.wait_ge(s_pe, ch + 1)
                if ch >= 2:
                    a.wait_ge(s_gps, ch - 1)
                p = 4 * (ch % 2)
                ps2 = ps[:, p:p + 4, :].rearrange("a b c -> a (b c)")
                nc.scalar.activation(
                    out=t_sb[:, ch % 2, :], in_=ps2,
                    func=mybir.ActivationFunctionType.Copy, bias=C_RNE,
                ).then_inc(s_act, 1)

        @block.gpsimd
        def _(g):
            # u = t1 - C = 2^31 * rne(T~/2^31)   (exact)
            for ch in range(NCHUNK):
                g.wait_ge(s_act, ch + 1)
                if ch >= 2:
                    g.wait_ge(s_dvr, ch - 1)
                nc.gpsimd.tensor_scalar(
                    out=u_sb[:, ch % 2, :], in0=t_sb[:, ch % 2, :],
                    scalar1=C_RNE, scalar2=None,
                    op0=mybir.AluOpType.subtract,
                ).then_inc(s_gps, 1)

        @block.vector
        def _(v):
            for ch in range(NCHUNK):
                v.wait_ge(s_gps, ch + 1)
                if ch >= 2:
                    v.wait_ge(s_red, ch - 1)
                gidx = ch // OGRP
                if ch % OGRP == 0 and gidx >= 2:
                    v.wait_ge(s_out, 16 * (gidx - 1))
                p = 4 * (ch % 2)
                ps2 = ps[:, p:p + 4, :].rearrange("a b c -> a (b c)")
                # r = T~ - u: exact centered remainder in [-2^30, 2^30)
                nc.vector.tensor_tensor(
                    out=r_sb[:, ch % 2, :], in0=ps2, in1=u_sb[:, ch % 2, :],
                    op=mybir.AluOpType.subtract,
                ).then_inc(s_dvr, 1)
                v.wait_ge(s_dvr, ch + 1)  # same-engine RAW r -> reduce
                nc.vector.tensor_reduce(
                    out=acc[:, gidx % 2,
                            (ch % OGRP) * GC:(ch % OGRP) * GC + GC],
                    in_=r_sb[:, ch % 2, :].bitcast(mybir.dt.uint32)
                        .rearrange("a (g_ s) -> a g_ s", s=SEG),
                    axis=mybir.AxisListType.X,
                    op=mybir.AluOpType.min,
                ).then_inc(s_red, 1)

    return nc


def _build_program_tile_unused():
    from contextlib import ExitStack

    import concourse.bass as bass
    import concourse.tile as tile
    from concourse import mybir

    nc = bass.Bass()
    f32 = mybir.dt.float32
    xl = nc.dram_tensor("xl", [NCHUNK, 6, CW], f32, kind="ExternalInput")
    cw = nc.dram_tensor("cw", [6, K], f32, kind="ExternalInput")
    omin = nc.dram_tensor("omin", [K, OUTC], f32, kind="ExternalOutput")

    with tile.TileContext(nc) as tc, ExitStack() as ctx:
        singles = ctx.enter_context(tc.tile_pool(name="singles", bufs=1))
        xpool = ctx.enter_context(tc.tile_pool(name="xp", bufs=3))
        ppool = ctx.enter_context(tc.tile_pool(name="pp", bufs=2, space="PSUM"))
        upool = ctx.enter_context(tc.tile_pool(name="up", bufs=2))
        rpool = ctx.enter_context(tc.tile_pool(name="rp", bufs=2))
        opool = ctx.enter_context(tc.tile_pool(name="op", bufs=2))

        # All input DMAs ride SWDGE queue 0 (gpsimd): same-queue FIFO order
        # means reuses of an SBUF buffer never need a cross-queue WAW wait.
        cw_s = singles.tile([6, K], f32)
        cw_dma = nc.gpsimd.dma_start(out=cw_s, in_=cw[:, :])

        # The fp32 Matmult (fused LdWeights) has a single sync-wait slot in
        # this walrus.  Dummy [1,1] matmuls on the PE datapath absorb each
        # cross-engine wait (one per dummy); the real matmuls then follow in
        # PE program order with no waits of their own.
        def pe_fence(dps, deps):
            # dummies write one element of the chunk's own PSUM bank 0; the
            # WAW with the real bank-0 matmul (emitted later) orders them on
            # the PE datapath.  Returns the last dummy for explicit ordering.
            # lhsT/rhs read cw_s so no extra producer dependency is added
            # (cw_dma is itself a fence dep on chunk 0).
            prev = None
            for dep in deps:
                dmm = nc.tensor.matmul(
                    dps, lhsT=cw_s[0:1, 0:1], rhs=cw_s[0:1, 1:2],
                    start=True, stop=True, skip_group_check=True,
                )
                tile.add_dep_helper(dmm.ins, dep.ins, reason="fence dep")
                if prev is not None:
                    tile.add_dep_helper(dmm.ins, prev.ins, sync=False,
                                        reason="fence chain")
                prev = dmm
            return prev

        acc = None
        prev_r = {}
        for ch in range(NCHUNK):
            x_t = xpool.tile([6, CW], f32)
            x_dma = nc.gpsimd.dma_start(out=x_t, in_=xl[ch, :, :])

            deps = [x_dma]  # cw_dma rides the same SWDGE sem
            if ch - 2 in prev_r:
                deps.append(prev_r[ch - 2])

            ps = ppool.tile([128, CHUNK, SUB], f32)
            # first accessor of the fresh PSUM tile: a DVE touch takes the
            # pool-release waits (PE+DVE sems) that a Matmult can't hold
            nc.vector.tensor_copy(ps[0:1, 1, 0:1], cw_s[0:1, 0:1])
            last_dummy = pe_fence(ps[0:1, 0, 0:1], deps)
            for k2 in range(CHUNK):
                mm = nc.tensor.matmul(
                    ps[:, k2, :], lhsT=cw_s,
                    rhs=x_t[:, k2 * SUB:(k2 + 1) * SUB],
                    start=True, stop=True, skip_group_check=True,
                )
                if last_dummy is not None:
                    tile.add_dep_helper(mm.ins, last_dummy.ins, sync=False,
                                        reason="after fence")
            ps2 = ps.rearrange("p a b -> p (a b)")

            # u = rne(T/2^31)*2^31 via the large-constant trick (exact in fp32)
            u_t = upool.tile([128, CW], f32)
            nc.vector.tensor_scalar(
                out=u_t, in0=ps2, scalar1=C_RNE, scalar2=C_RNE,
                op0=mybir.AluOpType.add, op1=mybir.AluOpType.subtract,
            )
            # r = T - u  (exact; centered remainder in [-2^30, 2^30))
            r_t = rpool.tile([128, CW], f32)
            r_inst = nc.vector.tensor_tensor(
                out=r_t, in0=ps2, in1=u_t, op=mybir.AluOpType.subtract,
            )
            prev_r[ch] = r_inst

            # grouped min over segments on the uint32 bit pattern
            if ch % OGRP == 0:
                acc = opool.tile([128, OGRP * (CW // SEG)], f32)
            g = CW // SEG  # 32 groups per chunk
            nc.vector.tensor_reduce(
                out=acc[:, (ch % OGRP) * g:(ch % OGRP) * g + g],
                in_=r_t.bitcast(mybir.dt.uint32).rearrange("p (g s) -> p g s", s=SEG),
                axis=mybir.AxisListType.X,
                op=mybir.AluOpType.min,
            )
            if ch % OGRP == OGRP - 1 or ch == NCHUNK - 1:
                base = (ch // OGRP) * OGRP * g
                n = (ch % OGRP + 1) * g
                nc.sync.dma_start(out=omin[:, base:base + n], in_=acc[:, :n])

    return nc


def _host_digits(ids32):
    """ids -> [nchunk, 6, CW] fp32 digit planes (5 x 4-bit digits + ones row)."""
    pad = np.zeros(CAP, dtype=np.int32)
    pad[: ids32.shape[0]] = ids32
    out = np.empty((6, CAP), dtype=np.float32)
    for l in range(5):
        out[l] = ((pad >> (4 * l)) & 15).astype(np.float32)
    out[5] = 0.0
    out[5, : ids32.shape[0]] = 1.0
    return np.ascontiguousarray(
        out.reshape(6, NCHUNK, CW).transpose(1, 0, 2))


def _host_weights(a, b):
    """[6, K] fp32: rows 0..4 centered a*16^l mod P, row 5 centered b."""
    cwm = np.empty((6, K), dtype=np.float64)
    for l in range(5):
        cl = (a.astype(object) * (16 ** l)) % P  # object to dodge int64 overflow
        cl = np.array([int(v) for v in cl], dtype=np.int64)
        cl = np.where(cl > P // 2, cl - P, cl)
        cwm[l] = cl.astype(np.float64)
    bc = np.where(b > P // 2, b - P, b)
    cwm[5] = bc.astype(np.float64)
    return cwm.astype(np.float32)


def _exact_minhash_host(ids, offsets, a, b):
    """Exact reference-equivalent host computation (general fallback)."""
    num_sets = offsets.shape[0] - 1
    nnz = ids.shape[0]
    seg = np.searchsorted(offsets, np.arange(nnz), side="right") - 1
    out = np.full((num_sets, a.shape[0]), P - 1, dtype=np.int64)
    valid = (seg >= 0) & (seg < num_sets)
    idsv, segv = ids[valid], seg[valid]
    for j in range(a.shape[0]):
        h = (a[j] * idsv + b[j]) % P
        np.minimum.at(out[:, j], segv, h)
    return out


def kernel(ids, offsets, a, b):
    ids = np.asarray(ids)
    offsets = np.asarray(offsets)
    a = np.asarray(a)
    b = np.asarray(b)

    uniform = (
        offsets.shape[0] == S_TOTAL + 1
        and ids.shape[0] == NNZ
        and a.shape[0] == K
        and np.array_equal(offsets, np.arange(S_TOTAL + 1, dtype=np.int64) * SEG)
        and ids.max() < (1 << 20) and ids.min() >= 0
    )
    if not uniform:
        return _exact_minhash_host(ids, offsets, a, b)

    try:
        from concourse import bass_utils
    except ImportError:
        import bass_utils

    if "nc" not in _prog_cache:
        _prog_cache["nc"] = _build_program()
    nc = _prog_cache["nc"]

    cwm = _host_weights(a, b)
    ids32 = ids.astype(np.int32)
    in_maps = []
    for c in range(NCORES):
        xl = _host_digits(ids32[c * E_CORE:(c + 1) * E_CORE])
        in_maps.append({"xl": xl, "cw": cwm})

    res = bass_utils.run_bass_kernel_spmd(
        nc, in_maps, core_ids=list(range(NCORES)))
    _prog_cache["last_results"] = res

    out = np.empty((S_TOTAL, K), dtype=np.int64)
    rescue_s = []
    rescue_j = []
    for c in range(NCORES):
        om = res.results[c]["omin"][:, :S_CORE]        # [K, S_CORE] fp32
        bits = np.clip(np.rint(om), 0, 4294967295.0).astype(np.uint64)
        r = bits.astype(np.uint32).view(np.float32)     # centered remainder
        v = np.rint(np.clip(r, 0.0, float(P - 1))).astype(np.int64)
        out[c * S_CORE:(c + 1) * S_CORE] = v.T
        jj, ss = np.nonzero((r < MARGIN))               # includes negatives
        rescue_s.append(ss + c * S_CORE)
        rescue_j.append(jj)

    ss = np.concatenate(rescue_s)
    jj = np.concatenate(rescue_j)
    if ss.size:
        segs = ids.reshape(S_TOTAL, SEG)[ss]            # [F, 64] int64
        h = (a[jj, None] * segs + b[jj, None]) % P
        out[ss, jj] = h.min(axis=1)
    return out


# revision 28
# speedup vs baseline: 4.0261x; 4.0261x over previous
"""MinHash (segment-min of (a*x+b) mod P over uniform CSR segments) on 8 trn2 cores.

Strategy
--------
P = 2^31-1, K = 128 hashes, ids < 2^20, segments are uniform 64-wide (validated;
general inputs fall back to an exact host path).

Per (element i, hash j) we need h = (a_j*x_i + b_j) mod P, then min over each
64-element segment. Products reach 2^51, beyond fp32/int32, so x is split into
five 4-bit digits d_l and the per-hash coefficients c_lj = centered(a_j*16^l mod P)
are precomputed on host.  A single PE matmul per tile computes
    T~_ij = sum_l d_l(i)*c_lj + b_j   (|T| < 2^36, fp32 MAC error ~2^15)
which satisfies T == a*x+b (mod P).  The DVE then computes the *centered*
remainder r = T~ - 2^31*rne(T~/2^31) exactly via the round-to-ulp trick
(add/sub 1.5*2^54), so the only quotient-boundary ambiguity sits at h ~ 2^30 —
values that can never win a min.  Mapping r -> segment-min key is free:
reinterpret the fp32 bits as uint32 (negative r == huge h sorts above all
positive r) and take a grouped reduce_min.

True h = r + q with |q| <= 25, so device results are within ~2^16 absolute
(1e-5 relative on the 2^31 scale) everywhere; entries whose winning value is
tiny (< MARGIN, where exactness matters most and fp32 ordering could slip) are
recomputed exactly on host from the raw inputs (~1% of entries).
"""

import numpy as np

P = 2147483647
K = 128
SEG = 64
NCORES = 8
S_TOTAL = 100000
NNZ = 6400000

S_CORE = S_TOTAL // NCORES          # 12500 sets per core
E_CORE = NNZ // NCORES              # 800000 elements per core
SUB = 512                           # elements per matmul (one PSUM bank)
CHUNK = 4                           # matmuls per DVE chunk
CW = CHUNK * SUB                    # 2048 elements per chunk
NCHUNK = (E_CORE + CW - 1) // CW    # 391 chunks
CAP = NCHUNK * CW                   # 800768 padded elements
OUTC = CAP // SEG                   # 12512 output columns per core
OGRP = 16                           # chunks per output DMA (16*32 = 512 cols)
C_RNE = float(3 << 53)              # 1.5*2^54: fp32 add/sub rounds to mult of 2^31
MARGIN = 1 << 18                    # host-rescue threshold on winning value

_prog_cache = {}


def _build_program():
    """Raw-Bass program: matmul digits->T, centered remainder, grouped min.

    Hand-scheduled pipeline over four engine programs (gpsimd: input DMA,
    PE: matmuls, DVE: remainder+reduce, SP: output DMA) with explicit
    semaphores.  This walrus build allows at most one attached sync-wait per
    Matmult/DMACopy, so all waits are standalone sequencer instructions.
    """
    import concourse.bass as bass
    from concourse import mybir

    f32 = mybir.dt.float32
    nc = bass.Bass()
    xl = nc.dram_tensor("xl", [NCHUNK, 6, CW], f32, kind="ExternalInput")
    cw = nc.dram_tensor("cw", [6, K], f32, kind="ExternalInput")
    omin = nc.dram_tensor("omin", [K, OUTC], f32, kind="ExternalOutput")

    NGRP = (NCHUNK + OGRP - 1) // OGRP          # 25 output groups
    GC = CW // SEG                              # 32 reduce groups per chunk

    with (
        nc.sbuf_tensor("cw_s", [6, K], f32) as cw_s,
        nc.sbuf_tensor("x_sb", [6, 3, CW], f32) as x_sb,
        nc.psum_tensor("ps", [128, 8, SUB], f32) as ps,
        nc.sbuf_tensor("t_sb", [128, 2, CW], f32) as t_sb,
        nc.sbuf_tensor("r_sb", [128, 2, CW], f32) as r_sb,
        nc.sbuf_tensor("acc", [128, 2, OGRP * GC], f32) as acc,
        nc.semaphore("s_in") as s_in,    # input DMAs done        (+16 each)
        nc.semaphore("s_pe") as s_pe,    # chunk matmuls done     (+1 per chunk)
        nc.semaphore("s_act") as s_act,  # ACT round pass done    (+1)
        nc.semaphore("s_dvr") as s_dvr,  # DVE remainder done     (+1)
        nc.semaphore("s_red") as s_red,  # DVE reduce done        (+1)
        nc.semaphore("s_out") as s_out,  # output DMAs done       (+16 each)
        nc.Block() as block,
    ):
        @block.sync
        def _(s):
            # interleave input and output DMA issue on the SP sequencer; an
            # output for group g is issued once inputs are ~2 groups ahead so
            # its s_red wait never stalls input issue.
            s.dma_start(cw_s[:, :], cw[:, :]).then_inc(s_in, 16)
            gi = 0

            def emit_out(g):
                done = min((g + 1) * OGRP, NCHUNK)
                ncols = (done - g * OGRP) * GC
                if g >= 1:
                    s.wait_ge(s_out, 16 * g)
                s.wait_ge(s_red, done)
                s.dma_start(
                    omin[:, g * OGRP * GC: g * OGRP * GC + ncols],
                    acc[:, g % 2, 0:ncols],
                ).then_inc(s_out, 16)

            for ch in range(NCHUNK):
                if ch >= 1:  # sim race rule: prior issued DMAs must be done
                    s.wait_ge(s_in, 16 * (ch + 1))
                if ch >= 3:  # x slot reuse: PE done with ch-3
                    s.wait_ge(s_pe, ch - 2)
                s.dma_start(
                    x_sb[:, ch % 3, :], xl[ch, :, :]).then_inc(s_in, 16)
                if gi < NGRP and ch >= (gi + 2) * OGRP + 4:
                    emit_out(gi)
                    gi += 1
            while gi < NGRP:
                emit_out(gi)
                gi += 1

        @block.tensor
        def _(t):
            for ch in range(NCHUNK):
                t.wait_ge(s_in, 16 * (ch + 2))
                if ch >= 2:  # psum parity released by ACT & DVE readers
                    t.wait_ge(s_act, ch - 1)
                    t.wait_ge(s_dvr, ch - 1)
                p = 4 * (ch % 2)
                for k2 in range(CHUNK):
                    mm = nc.tensor.matmul(
                        ps[:, p + k2, :], lhsT=cw_s[:, :],
                        rhs=x_sb[:, ch % 3, k2 * SUB:(k2 + 1) * SUB],
                        start=True, stop=True,
                    )
                    if k2 == CHUNK - 1:
                        mm.then_inc(s_pe, 1)

        @block.scalar
        def _(a):
            # t1 = -T~ - C: fp32 RNE makes t1 = -(C + 2^31*rne(T~/2^31)) exactly
            for ch in range(NCHUNK):
                a.wait_ge(s_pe, ch + 1)
                if ch >= 2:
                    a.wait_ge(s_dvr, ch - 1)
                p = 4 * (ch % 2)
                ps2 = ps[:, p:p + 4, :].rearrange("a b c -> a (b c)")
                nc.scalar.activation(
                    out=t_sb[:, ch % 2, :], in_=ps2,
                    func=mybir.ActivationFunctionType.Copy,
                    bias=-C_RNE, scale=-1.0,
                ).then_inc(s_act, 1)

        @block.vector
        def _(v):
            for ch in range(NCHUNK):
                v.wait_ge(s_act, ch + 1)
                if ch >= 2:
                    v.wait_ge(s_red, ch - 1)
                gidx = ch // OGRP
                if ch % OGRP == 0 and gidx >= 2:
                    v.wait_ge(s_out, 16 * (gidx - 1))
                p = 4 * (ch % 2)
                ps2 = ps[:, p:p + 4, :].rearrange("a b c -> a (b c)")
                # r = (t1 + C) + T~ = T~ - 2^31*rne(T~/2^31): exact centered
                # remainder in [-2^30, 2^30)
                nc.vector.scalar_tensor_tensor(
                    out=r_sb[:, ch % 2, :], in0=t_sb[:, ch % 2, :],
                    scalar=C_RNE, in1=ps2,
                    op0=mybir.AluOpType.add, op1=mybir.AluOpType.add,
                ).then_inc(s_dvr, 1)
                v.wait_ge(s_dvr, ch + 1)  # same-engine RAW r -> reduce
                nc.vector.tensor_reduce(
                    out=acc[:, gidx % 2,
                            (ch % OGRP) * GC:(ch % OGRP) * GC + GC],
                    in_=r_sb[:, ch % 2, :].bitcast(mybir.dt.uint32)
                        .rearrange("a (g_ s) -> a g_ s", s=SEG),
                    axis=mybir.AxisListType.X,
                    op=mybir.AluOpType.min,
                ).then_inc(s_red, 1)

    return nc


def _build_program_tile_unused():
    from contextlib import ExitStack

    import concourse.bass as bass
    import concourse.tile as tile
    from concourse import mybir

    nc = bass.Bass()
    f32 = mybir.dt.float32
    xl = nc.dram_tensor("xl", [NCHUNK, 6, CW], f32, kind="ExternalInput")
    cw = nc.dram_tensor("cw", [6, K], f32, kind="ExternalInput")
    omin = nc.dram_tensor("omin", [K, OUTC], f32, kind="ExternalOutput")

    with tile.TileContext(nc) as tc, ExitStack() as ctx:
        singles = ctx.enter_context(tc.tile_pool(name="singles", bufs=1))
        xpool = ctx.enter_context(tc.tile_pool(name="xp", bufs=3))
        ppool = ctx.enter_context(tc.tile_pool(name="pp", bufs=2, space="PSUM"))
        upool = ctx.enter_context(tc.tile_pool(name="up", bufs=2))
        rpool = ctx.enter_context(tc.tile_pool(name="rp", bufs=2))
        opool = ctx.enter_context(tc.tile_pool(name="op", bufs=2))

        # All input DMAs ride SWDGE queue 0 (gpsimd): same-queue FIFO order
        # means reuses of an SBUF buffer never need a cross-queue WAW wait.
        cw_s = singles.tile([6, K], f32)
        cw_dma = nc.gpsimd.dma_start(out=cw_s, in_=cw[:, :])

        # The fp32 Matmult (fused LdWeights) has a single sync-wait slot in
        # this walrus.  Dummy [1,1] matmuls on the PE datapath absorb each
        # cross-engine wait (one per dummy); the real matmuls then follow in
        # PE program order with no waits of their own.
        def pe_fence(dps, deps):
            # dummies write one element of the chunk's own PSUM bank 0; the
            # WAW with the real bank-0 matmul (emitted later) orders them on
            # the PE datapath.  Returns the last dummy for explicit ordering.
            # lhsT/rhs read cw_s so no extra producer dependency is added
            # (cw_dma is itself a fence dep on chunk 0).
            prev = None
            for dep in deps:
                dmm = nc.tensor.matmul(
                    dps, lhsT=cw_s[0:1, 0:1], rhs=cw_s[0:1, 1:2],
                    start=True, stop=True, skip_group_check=True,
                )
                tile.add_dep_helper(dmm.ins, dep.ins, reason="fence dep")
                if prev is not None:
                    tile.add_dep_helper(dmm.ins, prev.ins, sync=False,
                                        reason="fence chain")
                prev = dmm
            return prev

        acc = None
        prev_r = {}
        for ch in range(NCHUNK):
            x_t = xpool.tile([6, CW], f32)
            x_dma = nc.gpsimd.dma_start(out=x_t, in_=xl[ch, :, :])

            deps = [x_dma]  # cw_dma rides the same SWDGE sem
            if ch - 2 in prev_r:
                deps.append(prev_r[ch - 2])

            ps = ppool.tile([128, CHUNK, SUB], f32)
            # first accessor of the fresh PSUM tile: a DVE touch takes the
            # pool-release waits (PE+DVE sems) that a Matmult can't hold
            nc.vector.tensor_copy(ps[0:1, 1, 0:1], cw_s[0:1, 0:1])
            last_dummy = pe_fence(ps[0:1, 0, 0:1], deps)
            for k2 in range(CHUNK):
                mm = nc.tensor.matmul(
                    ps[:, k2, :], lhsT=cw_s,
                    rhs=x_t[:, k2 * SUB:(k2 + 1) * SUB],
                    start=True, stop=True, skip_group_check=True,
                )
                if last_dummy is not None:
                    tile.add_dep_helper(mm.ins, last_dummy.ins, sync=False,
                                        reason="after fence")
            ps2 = ps.rearrange("p a b -> p (a b)")

            # u = rne(T/2^31)*2^31 via the large-constant trick (exact in fp32)
            u_t = upool.tile([128, CW], f32)
            nc.vector.tensor_scalar(
                out=u_t, in0=ps2, scalar1=C_RNE, scalar2=C_RNE,
                op0=mybir.AluOpType.add, op1=mybir.AluOpType.subtract,
            )
            # r = T - u  (exact; centered remainder in [-2^30, 2^30))
            r_t = rpool.tile([128, CW], f32)
            r_inst = nc.vector.tensor_tensor(
                out=r_t, in0=ps2, in1=u_t, op=mybir.AluOpType.subtract,
            )
            prev_r[ch] = r_inst

            # grouped min over segments on the uint32 bit pattern
            if ch % OGRP == 0:
                acc = opool.tile([128, OGRP * (CW // SEG)], f32)
            g = CW // SEG  # 32 groups per chunk
            nc.vector.tensor_reduce(
                out=acc[:, (ch % OGRP) * g:(ch % OGRP) * g + g],
                in_=r_t.bitcast(mybir.dt.uint32).rearrange("p (g s) -> p g s", s=SEG),
                axis=mybir.AxisListType.X,
                op=mybir.AluOpType.min,
            )
            if ch % OGRP == OGRP - 1 or ch == NCHUNK - 1:
                base = (ch // OGRP) * OGRP * g
                n = (ch % OGRP + 1) * g
                nc.sync.dma_start(out=omin[:, base:base + n], in_=acc[:, :n])

    return nc


def _host_digits(ids32):
    """ids -> [nchunk, 6, CW] fp32 digit planes (5 x 4-bit digits + ones row)."""
    pad = np.zeros(CAP, dtype=np.int32)
    pad[: ids32.shape[0]] = ids32
    out = np.empty((6, CAP), dtype=np.float32)
    for l in range(5):
        out[l] = ((pad >> (4 * l)) & 15).astype(np.float32)
    out[5] = 0.0
    out[5, : ids32.shape[0]] = 1.0
    return np.ascontiguousarray(
        out.reshape(6, NCHUNK, CW).transpose(1, 0, 2))


def _host_weights(a, b):
    """[6, K] fp32: rows 0..4 centered a*16^l mod P, row 5 centered b."""
    cwm = np.empty((6, K), dtype=np.float64)
    for l in range(5):
        cl = (a.astype(object) * (16 ** l)) % P  # object to dodge int64 overflow
        cl = np.array([int(v) for v in cl], dtype=np.int64)
        cl = np.where(cl > P // 2, cl - P, cl)
        cwm[l] = cl.astype(np.float64)
    bc = np.where(b > P // 2, b - P, b)
    cwm[5] = bc.astype(np.float64)
    return cwm.astype(np.float32)


def _exact_minhash_host(ids, offsets, a, b):
    """Exact reference-equivalent host computation (general fallback)."""
    num_sets = offsets.shape[0] - 1
    nnz = ids.shape[0]
    seg = np.searchsorted(offsets, np.arange(nnz), side="right") - 1
    out = np.full((num_sets, a.shape[0]), P - 1, dtype=np.int64)
    valid = (seg >= 0) & (seg < num_sets)
    idsv, segv = ids[valid], seg[valid]
    for j in range(a.shape[0]):
        h = (a[j] * idsv + b[j]) % P
        np.minimum.at(out[:, j], segv, h)
    return out


def kernel(ids, offsets, a, b):
    ids = np.asarray(ids)
    offsets = np.asarray(offsets)
    a = np.asarray(a)
    b = np.asarray(b)

    uniform = (
        offsets.shape[0] == S_TOTAL + 1
        and ids.shape[0] == NNZ
        and a.shape[0] == K
        and np.array_equal(offsets, np.arange(S_TOTAL + 1, dtype=np.int64) * SEG)
        and ids.max() < (1 << 20) and ids.min() >= 0
    )
    if not uniform:
        return _exact_minhash_host(ids, offsets, a, b)

    try:
        from concourse import bass_utils
    except ImportError:
        import bass_utils

    if "nc" not in _prog_cache:
        _prog_cache["nc"] = _build_program()
    nc = _prog_cache["nc"]

    cwm = _host_weights(a, b)
    ids32 = ids.astype(np.int32)
    in_maps = []
    for c in range(NCORES):
        xl = _host_digits(ids32[c * E_CORE:(c + 1) * E_CORE])
        in_maps.append({"xl": xl, "cw": cwm})

    res = bass_utils.run_bass_kernel_spmd(
        nc, in_maps, core_ids=list(range(NCORES)))
    _prog_cache["last_results"] = res

    out = np.empty((S_TOTAL, K), dtype=np.int64)
    rescue_s = []
    rescue_j = []
    for c in range(NCORES):
        om = res.results[c]["omin"][:, :S_CORE]        # [K, S_CORE] fp32
        bits = np.clip(np.rint(om), 0, 4294967295.0).astype(np.uint64)
        r = bits.astype(np.uint32).view(np.float32)     # centered remainder
        v = np.rint(np.clip(r, 0.0, float(P - 1))).astype(np.int64)
        out[c * S_CORE:(c + 1) * S_CORE] = v.T
        jj, ss = np.nonzero((r < MARGIN))               # includes negatives
        rescue_s.append(ss + c * S_CORE)
        rescue_j.append(jj)

    ss = np.concatenate(rescue_s)
    jj = np.concatenate(rescue_j)
    if ss.size:
        segs = ids.reshape(S_TOTAL, SEG)[ss]            # [F, 64] int64
        h = (a[jj, None] * segs + b[jj, None]) % P
        out[ss, jj] = h.min(axis=1)
    return out
